# revision 21
# baseline (speedup 1.0000x reference)
"""NT-Xent (SimCLR) loss on 8 trn2 NeuronCores.

Math (matches the jax reference):
    z  = concat(z_i, z_j)                  [2B, D], 2B=8192, D=256
    zn = z / ||z||                         (row-normalize)
    sim = zn @ zn.T                        [2B, 2B]
    logits = where(diag, -9e15, sim) / T
    loss = -mean(log_softmax(logits)[r, pair(r)])

Sharding: rows are split across 8 cores (1024 rows each).  Each core
receives the FULL z with its own rows permuted to the front, twice:
row-major partition-tiled ("z_pm"[p, n, d] = z_perm[n*128+p, d], for
norms and pair dots) and transposed ("zT"[d, c] = z_perm[c, d], the
matmul operand layout), plus the positive-pair rows for its block
("z_pair_pm").  The permutation makes the kernel perfectly SPMD: the
diagonal (self-similarity) of row-tile t always falls in columns
[128*t, 128*t+128), so one NEFF serves all 8 cores, no collectives.

Because logits are bounded by 1/T (cosine in [-1,1]), log-softmax uses
the FIXED shift m = 1/T — no max pass:
    lse_r   = 1/T + log( sum_c exp(sim_rc/T - 1/T) )   (diag masked out)
    loss_r  = lse_r - d_r/T,   d_r = zn_r . zn_pair(r)
Each core returns its [128, 8] tile of (log s_r - d_r/T); the host adds
1/T and takes the mean (in fp64).

Pipeline (per core): row chunks stream in; each is sum-squared (GpSimd
squares + DVE reduces) and rsqrt'd (DVE-only bit-trick + Newton — keeps
ACT free for exp and avoids act-table reloads).  The 1/norm vector is
PE-transposed, bounced through DRAM, and broadcast-loaded across
partitions so zn.T columns are normalized with two wide DVE multiplies
per 2048-column group (writing fp32r).  Each group then runs 16 fp32r
matmuls (Gram block vs all 1024 of this core's rows) and one fused
exp-row-sum ACT pass straight out of PSUM.  fp32r Gram measured ~7e-5
max rel error — ample here.
"""

import numpy as np

B = 4096
D = 256
N = 2 * B            # 8192 rows total
P = 128              # SBUF partitions
NCORES = 8
RPC = N // NCORES    # 1024 rows per core
MT = RPC // P        # 8 row-tiles per core
NT = N // P          # 64 row-tiles total
CH = 8               # row-tiles per streamed chunk
NCH = NT // CH       # 8 chunks
NGRP = 4             # 2048-column Gram groups
GW = N // NGRP       # 2048 columns per group
TEMP = 0.07
SCALE = 1.0 / TEMP
NEG = -1.0e6         # additive diagonal mask (pre-temperature)

MM_MODE = "f32r"

_CACHED_NC = None


def _build_nc():
    import concourse.bass as bass
    import concourse.bacc as bacc
    import concourse.mybir as mybir
    import concourse.tile as tile
    from concourse.masks import make_identity

    f32 = mybir.dt.float32
    f32r = mybir.dt.float32r
    i32 = mybir.dt.int32
    AF = mybir.ActivationFunctionType
    ALU = mybir.AluOpType
    X = mybir.AxisListType.X

    nc = bacc.Bacc(name="ntxent")
    zp = nc.dram_tensor("z_pm", [P, NT, D], f32, kind="ExternalInput")
    # f32r dtype end-to-end: the BIR verifier requires every producer of an
    # fp32r-matmul operand to round to fp32r (np layout is identical to f32).
    ztype = f32r if MM_MODE == "f32r" else f32
    zt = nc.dram_tensor("zT", [D, N], ztype, kind="ExternalInput")
    zq = nc.dram_tensor("z_pair_pm", [P, MT, D], f32, kind="ExternalInput")
    out = nc.dram_tensor("row_loss", [P, MT], f32, kind="ExternalOutput")

    with tile.TileContext(nc) as tc:
        with (
            tc.tile_pool(name="big", bufs=1) as big,
            tc.tile_pool(name="rows", bufs=3) as rows,
            tc.tile_pool(name="brp", bufs=2) as brp,
            tc.tile_pool(name="small", bufs=1) as small,
            tc.tile_pool(name="scr", bufs=4) as scr,
            tc.tile_pool(name="expool", bufs=3) as expool,
            tc.tile_pool(name="psp", bufs=2, space="PSUM") as psp,
            tc.tile_pool(name="drp", bufs=1, space="DRAM") as drp,
        ):
            znt = [
                big.tile([P, N], ztype, name=f"znt{k}", tag=f"znt{k}")
                for k in range(2)
            ]
            zmine = big.tile([P, MT, D], f32, tag="zmine")
            zpairt = big.tile([P, MT, D], f32, tag="zpairt")

            SS = small.tile([P, NT], f32)
            RN = small.tile([P, NT], f32)
            SSp = small.tile([P, MT], f32)
            RNp = small.tile([P, MT], f32)
            SUMS = small.tile([P, MT * NGRP], f32)
            Ssum = small.tile([P, MT], f32)
            Ddraw = small.tile([P, MT], f32)
            Dd = small.tile([P, MT], f32)
            LOGS = small.tile([P, MT], f32)
            LOSS = small.tile([P, MT], f32)
            ident = small.tile([P, P], f32)
            dmask = small.tile([P, P], f32)
            nbias = small.tile([P, 1], f32)
            c15 = small.tile([P, CH], f32)
            magic = small.tile([P, CH], i32)
            rnt_sb = small.tile([CH, P], f32)

            rn_dram = drp.tile([N], f32)

            nc.vector.memset(nbias[:], -SCALE)
            nc.vector.memset(c15[:], 1.5)
            nc.gpsimd.memset(magic[:], 0x5F3759DF)
            make_identity(nc, ident[:])
            nc.gpsimd.memset(dmask[:], 0.0)
            nc.gpsimd.affine_select(
                out=dmask[:], in_=dmask[:], compare_op=ALU.not_equal,
                fill=NEG, base=0, pattern=[[-1, P]], channel_multiplier=1,
            )

            def rsqrt_newton(ss, rn, w):
                """rn = 1/sqrt(ss): Quake-style int seed + 3 Newton steps,
                entirely on DVE (keeps ACT free for exp; sqrt/ln on ACT
                would each force a ~1.3us act-table switch)."""
                sh = scr.tile([P, w], i32, tag=f"rsA{w}")
                nc.vector.tensor_scalar(
                    out=sh[:], in0=ss.bitcast(i32), scalar1=1, scalar2=None,
                    op0=ALU.logical_shift_right,
                )
                nc.vector.tensor_tensor(
                    out=sh[:], in0=magic[:, :w], in1=sh[:], op=ALU.subtract
                )
                y0 = sh[:].bitcast(f32)
                t1 = scr.tile([P, w], f32, tag=f"rsC{w}")
                for step in range(3):
                    nc.vector.tensor_mul(out=t1[:], in0=y0, in1=y0)
                    nc.vector.tensor_mul(out=t1[:], in0=t1[:], in1=ss)
                    nc.vector.scalar_tensor_tensor(
                        out=t1[:], in0=t1[:], scalar=-0.5, in1=c15[:, :w],
                        op0=ALU.mult, op1=ALU.add,
                    )
                    dst = rn if step == 2 else y0
                    nc.vector.tensor_mul(out=dst, in0=y0, in1=t1[:])

            # ---- zT loads (independent, start immediately) -----------------
            for k in range(2):
                for g in range(NGRP):
                    nc.sync.dma_start(
                        out=znt[k][:, g * GW : (g + 1) * GW],
                        in_=zt[k * P : (k + 1) * P, g * GW : (g + 1) * GW],
                    )

            def mm_ap(t_, a, b):
                return t_[:, a:b]

            def gram_group(g):
                """Gram rows 0:1024 x cols [GW*g, GW*(g+1)), exp-summed."""
                for t in range(MT):
                    lhs = [mm_ap(znt[k], t * P, (t + 1) * P) for k in range(2)]
                    ps = psp.tile([P, GW], f32, tag="mm")
                    for k in range(2):
                        for q in range(4):
                            c0 = g * GW + q * 512
                            nc.tensor.matmul(
                                ps[:, q * 512 : (q + 1) * 512],
                                lhs[k],
                                mm_ap(znt[k], c0, c0 + 512),
                                start=(k == 0),
                                stop=(k == 1),
                            )
                    if g == 0:
                        off = t * P
                        nc.vector.tensor_add(
                            out=ps[:, off : off + P],
                            in0=ps[:, off : off + P],
                            in1=dmask[:],
                        )
                    es = expool.tile([P, GW], f32, tag="es")
                    nc.scalar.activation(
                        out=es[:], in_=ps[:], func=AF.Exp,
                        bias=nbias[:], scale=SCALE,
                        accum_out=SUMS[:, t * NGRP + g : t * NGRP + g + 1],
                    )

            # ---- streamed main pipeline ------------------------------------
            for c8 in range(NCH):
                rt = rows.tile([P, CH, D], f32, tag="rt")
                nc.sync.dma_start(out=rt[:], in_=zp[:, c8 * CH : (c8 + 1) * CH, :])
                for i in range(CH):
                    gi = c8 * CH + i
                    s = scr.tile([P, D], f32, tag="sq")
                    nc.gpsimd.tensor_mul(
                        out=s[:], in0=rt[:, i, :], in1=rt[:, i, :]
                    )
                    nc.vector.reduce_sum(
                        out=SS[:, gi : gi + 1], in_=s[:], axis=X
                    )
                rn_sl = RN[:, c8 * CH : (c8 + 1) * CH]
                rsqrt_newton(SS[:, c8 * CH : (c8 + 1) * CH], rn_sl, CH)
                # rn slice -> [CH, P] -> DRAM (c-order), for broadcast reload
                pt = psp.tile([P, 512], f32, tag="mm")
                nc.tensor.transpose(
                    out=pt[:CH, :P], in_=rn_sl, identity=ident[:]
                )
                nc.vector.tensor_copy(out=rnt_sb[:], in_=pt[:CH, :P])
                nc.sync.dma_start(
                    out=rn_dram[c8 * CH * P : (c8 + 1) * CH * P].rearrange(
                        "(t p) -> t p", p=P
                    ),
                    in_=rnt_sb[:],
                )
                if c8 % 2 == 1:
                    g = c8 // 2
                    # broadcast rn columns across partitions, normalize zn.T
                    br = brp.tile([P, GW], f32, tag="br")
                    nc.gpsimd.dma_start(
                        out=br[:],
                        in_=bass.AP(
                            tensor=rn_dram.tensor,
                            offset=rn_dram.offset + g * GW,
                            ap=[[0, P], [1, GW]],
                        ),
                    )
                    for k in range(2):
                        seg = znt[k][:, g * GW : (g + 1) * GW]
                        nc.vector.tensor_mul(
                            out=seg, in0=seg.bitcast(f32), in1=br[:]
                        )
                    gram_group(g)

            # ---- pair block: raw dots + pair norms (fills late gaps) -------
            nc.sync.dma_start(out=zmine[:], in_=zp[:, 0:MT, :])
            nc.sync.dma_start(out=zpairt[:], in_=zq[:])
            for i in range(MT):
                s = scr.tile([P, D], f32, tag="sq")
                nc.gpsimd.tensor_mul(
                    out=s[:], in0=zmine[:, i, :], in1=zpairt[:, i, :]
                )
                nc.vector.reduce_sum(out=Ddraw[:, i : i + 1], in_=s[:], axis=X)
                sp = scr.tile([P, D], f32, tag="sq")
                nc.gpsimd.tensor_mul(
                    out=sp[:], in0=zpairt[:, i, :], in1=zpairt[:, i, :]
                )
                nc.vector.reduce_sum(out=SSp[:, i : i + 1], in_=sp[:], axis=X)
            rsqrt_newton(SSp[:], RNp[:], MT)

            # ---- finalize: loss_r = log s_r - d_r/T ------------------------
            sums_v = SUMS[:].rearrange("p (t g) -> p t g", g=NGRP)
            nc.vector.reduce_sum(out=Ssum[:], in_=sums_v, axis=X)
            nc.scalar.activation(out=LOGS[:], in_=Ssum[:], func=AF.Ln)
            nc.vector.tensor_mul(out=Dd[:], in0=Ddraw[:], in1=RN[:, 0:MT])
            nc.vector.tensor_mul(out=Dd[:], in0=Dd[:], in1=RNp[:])
            nc.vector.scalar_tensor_tensor(
                out=LOSS[:], in0=Dd[:], scalar=-SCALE, in1=LOGS[:],
                op0=ALU.mult, op1=ALU.add,
            )
            nc.sync.dma_start(out=out[:], in_=LOSS[:])

    nc.finalize()
    return nc


def _get_nc():
    global _CACHED_NC
    if _CACHED_NC is None:
        _CACHED_NC = _build_nc()
    return _CACHED_NC


def _to_pm(a):
    """[R, D] row-major -> [128, R/128, D] partition-major."""
    r = a.shape[0]
    return np.ascontiguousarray(a.reshape(r // P, P, D).transpose(1, 0, 2))


def make_in_maps(z_i, z_j):
    z = np.concatenate(
        [np.asarray(z_i, dtype=np.float32), np.asarray(z_j, dtype=np.float32)], axis=0
    )
    in_maps = []
    for c in range(NCORES):
        s0, s1 = c * RPC, (c + 1) * RPC
        z_perm = np.concatenate([z[s0:s1], z[:s0], z[s1:]], axis=0)
        p0 = (s0 + B) % N
        in_maps.append(
            {
                "z_pm": _to_pm(z_perm),
                "zT": np.ascontiguousarray(z_perm.T),
                "z_pair_pm": _to_pm(z[p0 : p0 + RPC]),
            }
        )
    return in_maps


def finish(results):
    total = 0.0
    for r in results:
        total += float(np.sum(r["row_loss"].astype(np.float64)))
    return np.asarray(SCALE + total / N, dtype=np.float32)


_LDW_PATCHED = False


def _enable_ldw_opt():
    """bass_utils hardcodes --enable-ldw-opt=false; our Gram issues 4
    consecutive matmuls per stationary operand, and the redundant
    LDWEIGHTS reloads cost ~190ns per matmul.  Flip the flag."""
    global _LDW_PATCHED
    if _LDW_PATCHED:
        return
    import concourse.bass_utils as bu

    orig = bu.run_command

    def patched(argv, **kwargs):
        argv = [
            "--enable-ldw-opt=true" if a == "--enable-ldw-opt=false" else a
            for a in argv
        ]
        return orig(argv, **kwargs)

    bu.run_command = patched
    _LDW_PATCHED = True


def run_spmd(z_i, z_j, **kw):
    _enable_ldw_opt()
    from concourse.bass_utils import run_bass_kernel_spmd

    in_maps = make_in_maps(z_i, z_j)
    return run_bass_kernel_spmd(_get_nc(), in_maps, core_ids=list(range(NCORES)), **kw)


def kernel(z_i, z_j):
    res = run_spmd(z_i, z_j)
    return finish(res.results)


if __name__ == "__main__":
    rng = np.random.default_rng(0)
    zi = rng.standard_normal((B, D), dtype=np.float32)
    zj = rng.standard_normal((B, D), dtype=np.float32)
    print(kernel(zi, zj))


# revision 22
# speedup vs baseline: 1.3047x; 1.3047x over previous
"""NT-Xent (SimCLR) loss on 8 trn2 NeuronCores.

Math (matches the jax reference):
    z  = concat(z_i, z_j)                  [2B, D], 2B=8192, D=256
    zn = z / ||z||                         (row-normalize)
    sim = zn @ zn.T                        [2B, 2B]
    logits = where(diag, -9e15, sim) / T
    loss = -mean(log_softmax(logits)[r, pair(r)])

Sharding: rows are split across 8 cores (1024 rows each).  Each core
receives the FULL z with its own rows permuted to the front, in a
partition-major layout ("z_pm"[p, n, d] = z_perm[n*128+p, d]) so DMA
loads are contiguous, plus the positive-pair rows for its block
("z_pair_pm") and a tiny diag-mask constant ("diag_aux").  The
permutation makes the kernel perfectly SPMD: the diagonal
(self-similarity) of row-tile t always falls in columns
[128*t, 128*t+128), so one NEFF serves all 8 cores, no collectives.

Because logits are bounded by 1/T (cosine in [-1,1]), log-softmax uses
the FIXED shift m = 1/T — no max pass:
    lse_r   = 1/T + log( sum_c exp(sim_rc/T - 1/T) )   (diag masked out)
    loss_r  = lse_r - d_r/T,   d_r = zn_r . zn_pair(r)
Each core returns its [128, 8] tile of (log s_r - d_r/T); the host adds
1/T and takes the mean (in fp64).

Pipeline (per core): rows stream in 8 chunks of 8 row-tiles.  Per chunk:
GpSimd squares + one batched DVE reduce give row sum-squares; a DVE-only
bit-trick rsqrt (Quake seed + Newton — keeps ACT free for exp and avoids
act-table thrash) gives 1/norms; DVE normalizes in place; TensorE
transposes the chunk into zn.T (evac split DVE/ACT).  After every second
chunk, one 2048-column Gram group runs: 16 fp32r matmuls + a diag-mask
accumulate-matmul (from diag_aux, so no vector op sits between matmul
and exp), then one fused exp-row-sum ACT pass straight out of PSUM.
fp32r Gram measured ~7e-5 max rel error — ample here.
"""

import numpy as np

B = 4096
D = 256
N = 2 * B            # 8192 rows total
P = 128              # SBUF partitions
NCORES = 8
RPC = N // NCORES    # 1024 rows per core
MT = RPC // P        # 8 row-tiles per core
NT = N // P          # 64 row-tiles total
CH = 8               # row-tiles per streamed chunk
NCH = NT // CH       # 8 chunks
NGRP = 4             # 2048-column Gram groups
GW = N // NGRP       # 2048 columns per group
TEMP = 0.07
SCALE = 1.0 / TEMP
NEG = -1.0e6         # additive diagonal mask (pre-temperature)

MM_MODE = "f32r"

_CACHED_NC = None


def _build_nc():
    import concourse.bacc as bacc
    import concourse.mybir as mybir
    import concourse.tile as tile
    from concourse.masks import make_identity

    f32 = mybir.dt.float32
    f32r = mybir.dt.float32r
    i32 = mybir.dt.int32
    AF = mybir.ActivationFunctionType
    ALU = mybir.AluOpType
    X = mybir.AxisListType.X
    ztype = f32r if MM_MODE == "f32r" else f32

    nc = bacc.Bacc(name="ntxent")
    zp = nc.dram_tensor("z_pm", [P, NT, D], f32, kind="ExternalInput")
    zq = nc.dram_tensor("z_pair_pm", [P, MT, D], f32, kind="ExternalInput")
    # [:, 0:128] = I, [:, 128:256] = NEG * I   (host-precomputed constant)
    zdg = nc.dram_tensor("diag_aux", [P, 2 * P], ztype, kind="ExternalInput")
    out = nc.dram_tensor("row_loss", [P, MT], f32, kind="ExternalOutput")

    with tile.TileContext(nc) as tc:
        with (
            tc.tile_pool(name="big", bufs=1) as big,
            tc.tile_pool(name="rows", bufs=3) as rows,
            tc.tile_pool(name="small", bufs=1) as small,
            tc.tile_pool(name="scr", bufs=3) as scr,
            tc.tile_pool(name="expool", bufs=3) as expool,
            tc.tile_pool(name="psp", bufs=2, space="PSUM") as psp,
        ):
            znt = [
                big.tile([P, N], ztype, name=f"znt{k}", tag=f"znt{k}")
                for k in range(2)
            ]
            zmine = big.tile([P, MT, D], f32, tag="zmine")
            zpairt = big.tile([P, MT, D], f32, tag="zpairt")

            SS = small.tile([P, NT], f32)
            RN = small.tile([P, NT], f32)
            SSp = small.tile([P, MT], f32)
            RNp = small.tile([P, MT], f32)
            SUMS = small.tile([P, MT * NGRP], f32)
            Ssum = small.tile([P, MT], f32)
            Ddraw = small.tile([P, MT], f32)
            Dd = small.tile([P, MT], f32)
            LOGS = small.tile([P, MT], f32)
            LOSS = small.tile([P, MT], f32)
            ident = small.tile([P, P], f32)
            dgc = small.tile([P, 2 * P], ztype)
            nbias = small.tile([P, 1], f32)
            c15 = small.tile([P, CH], f32)
            magic = small.tile([P, CH], i32)

            nc.vector.memset(nbias[:], -SCALE)
            nc.vector.memset(c15[:], 1.5)
            nc.gpsimd.memset(magic[:], 0x5F3759DF)
            make_identity(nc, ident[:])
            nc.sync.dma_start(out=dgc[:], in_=zdg[:])

            def rsqrt_newton(ss, rn, w, nsteps=3):
                """rn = 1/sqrt(ss): Quake-style int seed + Newton steps,
                entirely on DVE (keeps ACT free for exp; sqrt/ln on ACT
                would each force a ~1.3us act-table switch)."""
                sh = scr.tile([P, w], i32, tag=f"rsA{w}")
                nc.vector.tensor_scalar(
                    out=sh[:], in0=ss.bitcast(i32), scalar1=1, scalar2=None,
                    op0=ALU.logical_shift_right,
                )
                nc.vector.tensor_tensor(
                    out=sh[:], in0=magic[:, :w], in1=sh[:], op=ALU.subtract
                )
                y0 = sh[:].bitcast(f32)
                t1 = scr.tile([P, w], f32, tag=f"rsC{w}")
                for step in range(nsteps):
                    nc.vector.tensor_mul(out=t1[:], in0=y0, in1=y0)
                    nc.vector.tensor_mul(out=t1[:], in0=t1[:], in1=ss)
                    nc.vector.scalar_tensor_tensor(
                        out=t1[:], in0=t1[:], scalar=-0.5, in1=c15[:, :w],
                        op0=ALU.mult, op1=ALU.add,
                    )
                    dst = rn if step == nsteps - 1 else y0
                    nc.vector.tensor_mul(out=dst, in0=y0, in1=t1[:])

            def gram_group(g):
                """Gram rows 0:1024 x cols [GW*g, GW*(g+1)), exp-summed."""
                for t in range(MT):
                    lhs = [znt[k][:, t * P : (t + 1) * P] for k in range(2)]
                    ps = psp.tile([P, GW], f32, tag="mm")
                    qstar = t // 4 if g == 0 else -1
                    for k in range(2):
                        for q in range(4):
                            c0 = g * GW + q * 512
                            nc.tensor.matmul(
                                ps[:, q * 512 : (q + 1) * 512],
                                lhs[k],
                                znt[k][:, c0 : c0 + 512],
                                start=(k == 0),
                                stop=(k == 1 and q != qstar),
                            )
                    if g == 0:
                        # diagonal mask via accumulate-matmul: += NEG * I
                        off = t * P
                        nc.tensor.matmul(
                            ps[:, off : off + P],
                            dgc[:, P : 2 * P],
                            dgc[:, 0:P],
                            start=False,
                            stop=True,
                        )
                    es = expool.tile([P, GW], f32, tag="es")
                    nc.scalar.activation(
                        out=es[:], in_=ps[:], func=AF.Exp,
                        bias=nbias[:], scale=SCALE,
                        accum_out=SUMS[:, t * NGRP + g : t * NGRP + g + 1],
                    )

            # ---- streamed main pipeline ------------------------------------
            for c8 in range(NCH):
                rt = rows.tile([P, CH, D], f32, tag="rt")
                nc.sync.dma_start(out=rt[:], in_=zp[:, c8 * CH : (c8 + 1) * CH, :])
                sq = scr.tile([P, CH, D], f32, tag="sq")
                for i in range(CH):
                    nc.gpsimd.tensor_mul(
                        out=sq[:, i, :], in0=rt[:, i, :], in1=rt[:, i, :]
                    )
                nc.vector.reduce_sum(
                    out=SS[:, c8 * CH : (c8 + 1) * CH], in_=sq[:], axis=X
                )
                rn_sl = RN[:, c8 * CH : (c8 + 1) * CH]
                rsqrt_newton(SS[:, c8 * CH : (c8 + 1) * CH], rn_sl, CH)
                for i in range(CH):
                    gi = c8 * CH + i
                    nc.vector.tensor_scalar_mul(
                        out=rt[:, i, :], in0=rt[:, i, :],
                        scalar1=RN[:, gi : gi + 1],
                    )
                # transpose the normalized chunk into zn.T (both K halves)
                for k in range(2):
                    for half in range(2):
                        pt = psp.tile([P, 512], f32, tag="mm")
                        for q in range(4):
                            i = 4 * half + q
                            nc.tensor.transpose(
                                out=pt[:, q * P : (q + 1) * P],
                                in_=rt[:, i, k * P : (k + 1) * P],
                                identity=ident[:],
                            )
                        c0 = (c8 * CH + 4 * half) * P
                        dst = znt[k][:, c0 : c0 + 512]
                        if (2 * k + half) % 2 == 0:
                            nc.vector.tensor_copy(out=dst, in_=pt[:])
                        else:
                            nc.scalar.copy(out=dst, in_=pt[:])
                if c8 % 2 == 1:
                    gram_group(c8 // 2)

            # ---- pair block: raw dots + pair norms (fills late gaps) -------
            nc.sync.dma_start(out=zmine[:], in_=zp[:, 0:MT, :])
            nc.sync.dma_start(out=zpairt[:], in_=zq[:])
            sq2 = scr.tile([P, MT, D], f32, tag="sq")
            for i in range(MT):
                nc.gpsimd.tensor_mul(
                    out=sq2[:, i, :], in0=zmine[:, i, :], in1=zpairt[:, i, :]
                )
            nc.vector.reduce_sum(out=Ddraw[:], in_=sq2[:], axis=X)
            sq3 = scr.tile([P, MT, D], f32, tag="sq")
            for i in range(MT):
                nc.gpsimd.tensor_mul(
                    out=sq3[:, i, :], in0=zpairt[:, i, :], in1=zpairt[:, i, :]
                )
            nc.vector.reduce_sum(out=SSp[:], in_=sq3[:], axis=X)
            rsqrt_newton(SSp[:], RNp[:], MT)

            # ---- finalize: loss_r = log s_r - d_r/T ------------------------
            sums_v = SUMS[:].rearrange("p (t g) -> p t g", g=NGRP)
            nc.vector.reduce_sum(out=Ssum[:], in_=sums_v, axis=X)
            nc.scalar.activation(out=LOGS[:], in_=Ssum[:], func=AF.Ln)
            nc.vector.tensor_mul(out=Dd[:], in0=Ddraw[:], in1=RN[:, 0:MT])
            nc.vector.tensor_mul(out=Dd[:], in0=Dd[:], in1=RNp[:])
            nc.vector.scalar_tensor_tensor(
                out=LOSS[:], in0=Dd[:], scalar=-SCALE, in1=LOGS[:],
                op0=ALU.mult, op1=ALU.add,
            )
            nc.sync.dma_start(out=out[:], in_=LOSS[:])

    nc.finalize()
    return nc


def _get_nc():
    global _CACHED_NC
    if _CACHED_NC is None:
        _CACHED_NC = _build_nc()
    return _CACHED_NC


def _to_pm(a):
    """[R, D] row-major -> [128, R/128, D] partition-major."""
    r = a.shape[0]
    return np.ascontiguousarray(a.reshape(r // P, P, D).transpose(1, 0, 2))


def _diag_aux():
    m = np.zeros((P, 2 * P), dtype=np.float32)
    m[:, 0:P] = np.eye(P, dtype=np.float32)
    m[:, P : 2 * P] = NEG * np.eye(P, dtype=np.float32)
    return m


def make_in_maps(z_i, z_j):
    z = np.concatenate(
        [np.asarray(z_i, dtype=np.float32), np.asarray(z_j, dtype=np.float32)], axis=0
    )
    dga = _diag_aux()
    in_maps = []
    for c in range(NCORES):
        s0, s1 = c * RPC, (c + 1) * RPC
        z_perm = np.concatenate([z[s0:s1], z[:s0], z[s1:]], axis=0)
        p0 = (s0 + B) % N
        in_maps.append(
            {
                "z_pm": _to_pm(z_perm),
                "z_pair_pm": _to_pm(z[p0 : p0 + RPC]),
                "diag_aux": dga,
            }
        )
    return in_maps


def finish(results):
    total = 0.0
    for r in results:
        total += float(np.sum(r["row_loss"].astype(np.float64)))
    return np.asarray(SCALE + total / N, dtype=np.float32)


_LDW_PATCHED = False


def _enable_ldw_opt():
    """bass_utils hardcodes --enable-ldw-opt=false; our Gram issues 4
    consecutive matmuls per stationary operand, and the redundant
    LDWEIGHTS reloads cost ~190ns per matmul.  Flip the flag."""
    global _LDW_PATCHED
    if _LDW_PATCHED:
        return
    import concourse.bass_utils as bu

    orig = bu.run_command

    def patched(argv, **kwargs):
        argv = [
            "--enable-ldw-opt=true" if a == "--enable-ldw-opt=false" else a
            for a in argv
        ]
        return orig(argv, **kwargs)

    bu.run_command = patched
    _LDW_PATCHED = True


def run_spmd(z_i, z_j, **kw):
    _enable_ldw_opt()
    from concourse.bass_utils import run_bass_kernel_spmd

    in_maps = make_in_maps(z_i, z_j)
    return run_bass_kernel_spmd(_get_nc(), in_maps, core_ids=list(range(NCORES)), **kw)


def kernel(z_i, z_j):
    res = run_spmd(z_i, z_j)
    return finish(res.results)


if __name__ == "__main__":
    rng = np.random.default_rng(0)
    zi = rng.standard_normal((B, D), dtype=np.float32)
    zj = rng.standard_normal((B, D), dtype=np.float32)
    print(kernel(zi, zj))
